# revision 7
# baseline (speedup 1.0000x reference)
"""CCT dense-transformer kernel for 8 Trainium2 NeuronCores.

Sharding: the 8 sequence-chunk heads are independent end-to-end (the per-head
attention covers tokens [h*512:(h+1)*512], and every other op — layernorms,
out-projection, MLP — is per-token), so core h computes the full forward for
its 512-token slice of all 4 batches. No collectives.

Per-core math (per batch b, head h, tokens n=512, kv width CH=960):
  z    = LN(concat(x1..x4))                  (stats fp32, token-major)
  K=V  = z @ (g?(Wv@Wk)).T + 1?u.T           (host-composed + LN-affine fold)
  per branch i (c in 64,128,256,512):
    z1  = LN1(x_i)
    qh  = z1 @ Wq'.T          (Wq' = g1?Wq * 1/sqrt(960); bias via ones-row)
    aT  = K.T-contract: aT[k,c] = sum_n K[n,k] qh[n,c]
    inorm2d over map, softmax over k: expA = exp((aT-m)*rs); denom folded
      into Wo as column scales (softmax normalizer is per-c)
    oT  = sum_k expA[k,:] KT[k,:]  -> att = oT.T @ Wo'sc + z1   (residual)
    t   = LN2(att+z1);  out = gelu(t@Wf1'.T)@Wf2'.T + b + (att+z1)
Matmul operands bf16 (PSUM fp32 accum); stats/residual fp32.
"""

import os
import numpy as np

H, B, NSEQ, CH = 8, 4, 4096, 960
NTOK = 512  # tokens per head-chunk
TT = NTOK // 128  # token tiles
CS = [64, 128, 256, 512]
COFF = [0, 64, 192, 448]
KVCH = [(s * 128, min(128, CH - s * 128)) for s in range(8)]  # 7x128 + 64

LAST_EXEC_NS = [None]

_built = {}


def _qchunks(c):
    """K-chunk list (slot, rows) for a (c+1)-row operand (c data rows + ones row)."""
    out = []
    nfull = c // 128
    for s in range(nfull):
        out.append((s, 128))
    rem = c % 128
    out.append((nfull, rem + 1))
    return out


def _ones_pos(c):
    return (c % 128, c // 128)


def _build():
    if "nc" in _built:
        return _built["nc"]
    import concourse.bass as bass
    import concourse.mybir as mybir
    import concourse.tile as tile
    from concourse.vector_clock import ScopedClock
    from contextlib import ExitStack

    # ---- compat: this walrus build accepts at most ONE sync-wait per
    # instruction; split Tile's tail-drain waits and (later, after build)
    # every multi-wait instruction into single-wait NOPs.
    def _patched_drain_and_barrier(self, tick_clock, wait_clock):
        probe = self.nc.sync.nop(nofuse=True)
        wait_clock.add_sem_waits(probe.ins, ScopedClock({None: tick_clock.global_clock}))
        si = probe.ins.sync_info
        waits = list(si.on_wait) if si is not None else []
        if len(waits) > 1:
            si.on_wait = waits[:1]
            for w in waits[1:]:
                n2 = self.nc.sync.nop(nofuse=True)
                n2.ins.sync_info = mybir.SyncInfo(on_wait=[w], on_update=[])
        self.nc.sync.drain()
        self.nc.all_engine_barrier()
        assert self.sems is not None
        popped = self.nc._tile_sem_poison_stack.pop()
        assert popped is self._sem_poison
        self.nc.clear_and_free_semaphores(list(self.sems.allocated().values()))
        self.nc.all_engine_barrier()

    tile.TileContext._drain_and_barrier = _patched_drain_and_barrier

    f32 = mybir.dt.float32
    bf16 = mybir.dt.bfloat16
    AF = mybir.ActivationFunctionType
    ALU = mybir.AluOpType
    AX = mybir.AxisListType
    from concourse.bass_isa import ReduceOp

    nc = bass.Bass(num_devices=8, num_swdge_queues=4)

    x_d = nc.declare_dram_parameter("x", [B, NTOK, CH], f32, isOutput=False)
    out_d = nc.declare_dram_parameter("out", [B, NTOK, CH], f32, isOutput=True)
    wkv_d = nc.declare_dram_parameter("wkv", [8, 128, CH], bf16, isOutput=False)
    id_d = nc.declare_dram_parameter("ident", [128, 128], bf16, isOutput=False)
    wq_d, wo_d, wf1_d, wf2_d = [], [], [], []
    for i, c in enumerate(CS):
        nq = c // 128 + 1
        cch = max(1, c // 128)
        nh = 4 * c // 128
        wq_d.append(nc.declare_dram_parameter(f"wq{i}", [nq, 128, c], bf16, isOutput=False))
        wo_d.append(nc.declare_dram_parameter(f"wo{i}", [cch, 128, c], bf16, isOutput=False))
        wf1_d.append(nc.declare_dram_parameter(f"wf1{i}", [nq, 128, 4 * c], bf16, isOutput=False))
        wf2_d.append(nc.declare_dram_parameter(f"wf2{i}", [nh + 1, 128, c], bf16, isOutput=False))

    with tile.TileContext(nc) as tc, ExitStack() as ctx:
        consts = ctx.enter_context(tc.tile_pool(name="consts", bufs=1))
        wpool = ctx.enter_context(tc.tile_pool(name="wpool", bufs=1))
        xpool = ctx.enter_context(tc.tile_pool(name="xpool", bufs=2))
        zpool = ctx.enter_context(tc.tile_pool(name="zpool", bufs=1))
        kpool = ctx.enter_context(tc.tile_pool(name="kpool", bufs=1))
        bpool = ctx.enter_context(tc.tile_pool(name="bpool", bufs=1))
        spool = ctx.enter_context(tc.tile_pool(name="spool", bufs=4))
        scr = ctx.enter_context(tc.tile_pool(name="scr", bufs=4))
        scr2 = ctx.enter_context(tc.tile_pool(name="scr2", bufs=2))
        h1pool = ctx.enter_context(tc.tile_pool(name="h1pool", bufs=3))
        opool = ctx.enter_context(tc.tile_pool(name="opool", bufs=2))
        psA = ctx.enter_context(tc.tile_pool(name="psA", bufs=2, space="PSUM"))
        psT = ctx.enter_context(tc.tile_pool(name="psT", bufs=2, space="PSUM"))
        psO = ctx.enter_context(tc.tile_pool(name="psO", bufs=1, space="PSUM"))

        ident = consts.tile([128, 128], bf16)
        nc.gpsimd.dma_start(out=ident[:], in_=id_d[:])
        onescol = consts.tile([128, 1], bf16)
        nc.vector.memset(onescol, 1.0)
        ones128 = consts.tile([128, 128], f32)
        nc.vector.memset(ones128, 1.0)
        onesrow = consts.tile([1, 512], bf16)
        nc.vector.memset(onesrow, 1.0)
        epsln = consts.tile([128, 1], f32)
        nc.vector.memset(epsln, 1e-6)
        epsin = consts.tile([128, 1], f32)
        nc.vector.memset(epsin, 1e-5)

        # resident weights
        wkv_sb = wpool.tile([128, 8, CH], bf16)
        nc.gpsimd.dma_start(out=wkv_sb[:], in_=wkv_d.rearrange("s p c -> p s c"))
        wq_sb, wo_sb, wf1_sb, wf2_sb, wosc_sb = [], [], [], [], []
        for i, c in enumerate(CS):
            nq = c // 128 + 1
            cch = max(1, c // 128)
            nh = 4 * c // 128
            t = wpool.tile([128, nq, c], bf16, name=f"wq_sb{i}")
            nc.gpsimd.dma_start(out=t[:], in_=wq_d[i].rearrange("s p c -> p s c"))
            wq_sb.append(t)
            t = wpool.tile([128, cch, c], bf16, name=f"wo_sb{i}")
            nc.gpsimd.dma_start(out=t[:], in_=wo_d[i].rearrange("s p c -> p s c"))
            wo_sb.append(t)
            t = wpool.tile([128, nq, 4 * c], bf16, name=f"wf1_sb{i}")
            nc.gpsimd.dma_start(out=t[:], in_=wf1_d[i].rearrange("s p c -> p s c"))
            wf1_sb.append(t)
            t = wpool.tile([128, nh + 1, c], bf16, name=f"wf2_sb{i}")
            nc.gpsimd.dma_start(out=t[:], in_=wf2_d[i].rearrange("s p c -> p s c"))
            wf2_sb.append(t)
            wosc_sb.append(wpool.tile([128, cch, c], bf16, name=f"wosc_sb{i}"))

        def ln_stats(src_ap_fn, c, mt_tile, rs_tile, eps_tile):
            """Per-token LN stats over c channels; writes mean/rstd [128, TT]."""
            for t in range(TT):
                sub = 480 if c == CH else c
                nsub = c // sub
                st = spool.tile([128, nsub, 6], f32, name="bnst", tag="bnst")
                src = src_ap_fn(t)
                if nsub > 1:
                    sv = src.rearrange("p (s f) -> p s f", f=sub)
                    for s in range(nsub):
                        nc.vector.bn_stats(st[:, s, :], sv[:, s, :])
                else:
                    nc.vector.bn_stats(st[:, 0, :], src)
                mv = spool.tile([128, 2], f32, name="bnmv", tag="bnmv")
                nc.vector.bn_aggr(mv[:], st[:])
                nc.vector.tensor_copy(mt_tile[:, t : t + 1], mv[:, 0:1])
                sd = spool.tile([128, 1], f32, name="bnsd", tag="bnsd")
                nc.scalar.activation(sd[:], mv[:, 1:2], AF.Sqrt, bias=eps_tile[:])
                nc.vector.reciprocal(rs_tile[:, t : t + 1], sd[:])

        for b in range(B):
            xb = xpool.tile([128, TT, CH], f32)
            nc.gpsimd.dma_start(
                out=xb[:], in_=x_d[b].rearrange("(t p) c -> p t c", p=128)
            )

            mkv = spool.tile([128, TT], f32, name="mkv")
            rkv = spool.tile([128, TT], f32, name="rkv")
            ln_stats(lambda t: xb[:, t, :], CH, mkv, rkv, epsln)

            # z.T (kv-normalized, channel-major) with ones row for the bias fold
            zT = zpool.tile([128, 8, NTOK], bf16, name="zT")
            nc.gpsimd.memset(zT[64:65, 7, :], 1.0)
            for t in range(TT):
                for s, (cs0, cw) in enumerate(KVCH):
                    zb = scr.tile([128, 128], bf16, name="zscr", tag="zscr")
                    nc.vector.tensor_scalar(
                        out=zb[:, :cw],
                        in0=xb[:, t, cs0 : cs0 + cw],
                        scalar1=mkv[:, t : t + 1],
                        scalar2=rkv[:, t : t + 1],
                        op0=ALU.subtract,
                        op1=ALU.mult,
                    )
                    pt = psT.tile([128, 128], bf16, name="pst", tag="pst")
                    nc.tensor.transpose(pt[:cw, :], zb[:, :cw], ident[:])
                    nc.scalar.copy(zT[0:cw, s, t * 128 : (t + 1) * 128], pt[:cw, :])

            # K (token-major) = z @ Wkv'.T ; KT (channel-major) by transpose
            Kb = kpool.tile([128, TT, CH], bf16, name="Kb")
            for mt in range(TT):
                for nb0, nw in ((0, 512), (512, 448)):
                    pk = psA.tile([128, 512], f32, name="psa", tag="psa")
                    for s in range(8):
                        ks = 128 if s < 7 else 65
                        nc.tensor.matmul(
                            pk[:, :nw],
                            lhsT=zT[0:ks, s, mt * 128 : (mt + 1) * 128],
                            rhs=wkv_sb[0:ks, s, nb0 : nb0 + nw],
                            start=(s == 0),
                            stop=(s == 7),
                        )
                    nc.scalar.copy(Kb[:, mt, nb0 : nb0 + nw], pk[:, :nw])
            KTb = kpool.tile([128, 8, NTOK], bf16, name="KTb")
            for s, (cs0, cw) in enumerate(KVCH):
                for t in range(TT):
                    pt = psT.tile([128, 128], bf16, name="pst", tag="pst")
                    nc.tensor.transpose(pt[:cw, :], Kb[:, t, cs0 : cs0 + cw], ident[:])
                    nc.vector.tensor_copy(KTb[0:cw, s, t * 128 : (t + 1) * 128], pt[:cw, :])

            for i, c in enumerate(CS):
                co = COFF[i]
                cch = max(1, c // 128)
                orow, oslot = _ones_pos(c)
                m1 = spool.tile([128, TT], f32, name=f"m1_{i}", tag="m1")
                r1 = spool.tile([128, TT], f32, name=f"r1_{i}", tag="r1")
                ln_stats(lambda t: xb[:, t, co : co + c], c, m1, r1, epsln)

                # z1.T with ones row
                z1T = bpool.tile([128, c // 128 + 1, NTOK], bf16, name=f"z1T_{i}", tag=f"z1T{i}")
                nc.gpsimd.memset(z1T[orow : orow + 1, oslot, :], 1.0)
                for t in range(TT):
                    for s in range(cch):
                        cw = min(128, c - s * 128)
                        zb = scr.tile([128, 128], bf16, name="z1scr", tag="zscr")
                        nc.vector.tensor_scalar(
                            out=zb[:, :cw],
                            in0=xb[:, t, co + s * 128 : co + s * 128 + cw],
                            scalar1=m1[:, t : t + 1],
                            scalar2=r1[:, t : t + 1],
                            op0=ALU.subtract,
                            op1=ALU.mult,
                        )
                        pt = psT.tile([128, 128], bf16, name="pst", tag="pst")
                        nc.tensor.transpose(pt[:cw, :], zb[:, :cw], ident[:])
                        nc.scalar.copy(z1T[0:cw, s, t * 128 : (t + 1) * 128], pt[:cw, :])

                # qh = z1 @ Wq'.T  (token-major)
                qhb = bpool.tile([128, TT, c], bf16, name=f"qhb_{i}", tag="qhb")
                qck = _qchunks(c)
                for mt in range(TT):
                    pq = psA.tile([128, 512], f32, name="psa", tag="psa")
                    for j, (s, ks) in enumerate(qck):
                        nc.tensor.matmul(
                            pq[:, :c],
                            lhsT=z1T[0:ks, s, mt * 128 : (mt + 1) * 128],
                            rhs=wq_sb[i][0:ks, s, :],
                            start=(j == 0),
                            stop=(j == len(qck) - 1),
                        )
                    nc.scalar.copy(qhb[:, mt, :], pq[:, :c])

                # aT[k, c] with inorm stats collected on the fly
                aS = bpool.tile([128, 8, c], bf16, name=f"aS_{i}", tag="aS")
                rsum = spool.tile([128, 8], f32, name="rsum", tag="rsum")
                rsq = spool.tile([128, 8], f32, name="rsq", tag="rsq")
                nc.vector.memset(rsum[64:128, 7:8], 0.0)
                nc.vector.memset(rsq[64:128, 7:8], 0.0)
                for kt, (ks0, kw) in enumerate(KVCH):
                    pa = psA.tile([128, 512], f32, name="psa", tag="psa")
                    for mt in range(TT):
                        nc.tensor.matmul(
                            pa[:kw, :c],
                            lhsT=Kb[:, mt, ks0 : ks0 + kw],
                            rhs=qhb[:, mt, :],
                            start=(mt == 0),
                            stop=(mt == TT - 1),
                        )
                    nc.scalar.activation(
                        aS[0:kw, kt, :],
                        pa[:kw, :c],
                        AF.Copy,
                        accum_out=rsum[0:kw, kt : kt + 1],
                    )
                    sq = scr2.tile([128, 512], bf16, name="sqscr", tag="sqscr")
                    nc.scalar.activation(
                        sq[:kw, :c],
                        pa[:kw, :c],
                        AF.Square,
                        accum_out=rsq[0:kw, kt : kt + 1],
                    )
                # inorm scalars (replicated on all partitions)
                prs = psT.tile([128, 8], f32, name="prs", tag="pst")
                nc.tensor.matmul(prs[:], lhsT=ones128[:], rhs=rsum[:], start=True, stop=True)
                stot = spool.tile([128, 1], f32, name="stot", tag="stot")
                nc.vector.reduce_sum(stot[:], prs[:], axis=AX.X)
                prq = psT.tile([128, 8], f32, name="prq", tag="pst")
                nc.tensor.matmul(prq[:], lhsT=ones128[:], rhs=rsq[:], start=True, stop=True)
                qtot = spool.tile([128, 1], f32, name="qtot", tag="qtot")
                nc.vector.reduce_sum(qtot[:], prq[:], axis=AX.X)
                inv_m = 1.0 / (CH * c)
                mean = spool.tile([128, 1], f32, name="mean", tag="mean")
                nc.vector.tensor_scalar_mul(mean[:], stot[:], inv_m)
                msq = spool.tile([128, 1], f32, name="msq", tag="msq")
                nc.vector.tensor_mul(msq[:], mean[:], mean[:])
                var = spool.tile([128, 1], f32, name="var", tag="var")
                nc.vector.tensor_scalar(
                    out=var[:], in0=qtot[:], scalar1=inv_m, scalar2=None, op0=ALU.mult
                )
                nc.vector.tensor_sub(var[:], var[:], msq[:])
                sd = spool.tile([128, 1], f32, name="sdin", tag="sdin")
                nc.scalar.activation(sd[:], var[:], AF.Sqrt, bias=epsin[:])
                rstd = spool.tile([128, 1], f32, name="rstd", tag="rstd")
                nc.vector.reciprocal(rstd[:], sd[:])
                nbias = spool.tile([128, 1], f32, name="nbias", tag="nbias")
                nc.vector.tensor_mul(nbias[:], mean[:], rstd[:])
                nc.vector.tensor_scalar_mul(nbias[:], nbias[:], -1.0)

                # softmax numerator
                expA = bpool.tile([128, 8, c], bf16, name=f"expA_{i}", tag="expA")
                for kt, (ks0, kw) in enumerate(KVCH):
                    nc.scalar.activation(
                        expA[0:kw, kt, :],
                        aS[0:kw, kt, :],
                        AF.Exp,
                        bias=nbias[0:kw],
                        scale=rstd[0:kw],
                    )
                # denominators -> reciprocal column chunks -> scale Wo
                pd = psA.tile([128, 512], f32, name="pd", tag="psa")
                for kt, (ks0, kw) in enumerate(KVCH):
                    nc.tensor.matmul(
                        pd[0:1, :c],
                        lhsT=onescol[0:kw, :],
                        rhs=expA[0:kw, kt, :],
                        start=(kt == 0),
                        stop=(kt == 7),
                    )
                rrow = spool.tile([1, 512], f32, name="rrow", tag="rrow")
                nc.vector.reciprocal(rrow[0:1, :c], pd[0:1, :c])
                rcol = spool.tile([128, 4], f32, name="rcol", tag="rcol")
                for s in range(cch):
                    cw = min(128, c - s * 128)
                    nc.gpsimd.dma_start(
                        out=rcol[0:cw, s : s + 1],
                        in_=rrow[0:1, s * 128 : s * 128 + cw],
                    )
                for s in range(cch):
                    cw = min(128, c - s * 128)
                    nc.vector.tensor_scalar_mul(
                        wosc_sb[i][0:cw, s, :], wo_sb[i][0:cw, s, :], rcol[0:cw, s : s + 1]
                    )

                # oT[c, n] = sum_k expA[k, cslice] KT[k, :]
                oTb = bpool.tile([128, cch, NTOK], bf16, name=f"oTb_{i}", tag="oTb")
                for s2 in range(cch):
                    cw2 = min(128, c - s2 * 128)
                    po = psA.tile([128, 512], f32, name="psa", tag="psa")
                    for kt in range(8):
                        kw = KVCH[kt][1]
                        nc.tensor.matmul(
                            po[:cw2, :],
                            lhsT=expA[0:kw, kt, s2 * 128 : s2 * 128 + cw2],
                            rhs=KTb[0:kw, kt, :],
                            start=(kt == 0),
                            stop=(kt == 7),
                        )
                    nc.vector.tensor_copy(oTb[0:cw2, s2, :], po[:cw2, :])

                # att = oT.T @ Wo'sc + z1 (residual, recomputed from x)
                ab = bpool.tile([128, TT, c], f32, name=f"ab_{i}", tag="ab")
                for mt in range(TT):
                    pat = psA.tile([128, 512], f32, name="psa", tag="psa")
                    for s2 in range(cch):
                        cw2 = min(128, c - s2 * 128)
                        nc.tensor.matmul(
                            pat[:, :c],
                            lhsT=oTb[0:cw2, s2, mt * 128 : (mt + 1) * 128],
                            rhs=wosc_sb[i][0:cw2, s2, :],
                            start=(s2 == 0),
                            stop=(s2 == cch - 1),
                        )
                    zr = scr2.tile([128, 512], f32, name="zres", tag="zres")
                    nc.vector.tensor_scalar(
                        out=zr[:, :c],
                        in0=xb[:, mt, co : co + c],
                        scalar1=m1[:, mt : mt + 1],
                        scalar2=r1[:, mt : mt + 1],
                        op0=ALU.subtract,
                        op1=ALU.mult,
                    )
                    nc.vector.tensor_add(ab[:, mt, :], pat[:, :c], zr[:, :c])

                # LN2 -> t.T (with ones row)
                m2 = spool.tile([128, TT], f32, name="m2", tag="m2")
                r2 = spool.tile([128, TT], f32, name="r2", tag="r2")
                ln_stats(lambda t: ab[:, t, :], c, m2, r2, epsln)
                tT = bpool.tile([128, c // 128 + 1, NTOK], bf16, name=f"tT_{i}", tag=f"z1T{i}")
                nc.gpsimd.memset(tT[orow : orow + 1, oslot, :], 1.0)
                for t in range(TT):
                    for s in range(cch):
                        cw = min(128, c - s * 128)
                        zb = scr.tile([128, 128], bf16, name="tscr", tag="zscr")
                        nc.vector.tensor_scalar(
                            out=zb[:, :cw],
                            in0=ab[:, t, s * 128 : s * 128 + cw],
                            scalar1=m2[:, t : t + 1],
                            scalar2=r2[:, t : t + 1],
                            op0=ALU.subtract,
                            op1=ALU.mult,
                        )
                        pt = psT.tile([128, 128], bf16, name="pst", tag="pst")
                        nc.tensor.transpose(pt[:cw, :], zb[:, :cw], ident[:])
                        nc.scalar.copy(tT[0:cw, s, t * 128 : (t + 1) * 128], pt[:cw, :])

                # fc1 (gelu) chunk-streamed into fc2 accumulation
                psum_o = [psO.tile([128, 512], f32, name=f"pso{mt}", tag=f"pso{mt}") for mt in range(TT)]
                nh = 4 * c // 128
                for ft in range(nh):
                    ph = psA.tile([128, 512], f32, name="psa", tag="psa")
                    for j, (s, ks) in enumerate(qck):
                        nc.tensor.matmul(
                            ph[:, :],
                            lhsT=wf1_sb[i][0:ks, s, ft * 128 : (ft + 1) * 128],
                            rhs=tT[0:ks, s, :],
                            start=(j == 0),
                            stop=(j == len(qck) - 1),
                        )
                    h1 = h1pool.tile([128, 512], bf16, name="h1", tag="h1")
                    nc.scalar.activation(h1[:], ph[:], AF.Gelu)
                    for mt in range(TT):
                        nc.tensor.matmul(
                            psum_o[mt][:, :c],
                            lhsT=h1[:, mt * 128 : (mt + 1) * 128],
                            rhs=wf2_sb[i][0:128, ft, :],
                            start=(ft == 0),
                            stop=False,
                        )
                for mt in range(TT):
                    nc.tensor.matmul(
                        psum_o[mt][:, :c],
                        lhsT=onesrow[0:1, mt * 128 : (mt + 1) * 128],
                        rhs=wf2_sb[i][0:1, nh, :],
                        start=False,
                        stop=True,
                    )
                    ot = opool.tile([128, 512], f32, name="ot", tag="ot")
                    nc.vector.tensor_add(ot[:, :c], psum_o[mt][:, :c], ab[:, mt, :])
                    nc.gpsimd.dma_start(
                        out=out_d[b, mt * 128 : (mt + 1) * 128, co : co + c],
                        in_=ot[:, :c],
                    )

    # split multi-wait instructions for this walrus build
    n = 0
    ctr = [0]
    for func in nc.m.functions:
        for block in func.blocks:
            insts = block.instructions
            idx = 0
            while idx < len(insts):
                inst = insts[idx]
                si = inst.sync_info
                if si is not None and len(si.on_wait) > 1:
                    extra = list(si.on_wait[:-1])
                    si.on_wait = [si.on_wait[-1]]
                    for w in extra:
                        ctr[0] += 1
                        nop = mybir.InstNoOp(
                            name=f"I-waitsplit-{ctr[0]}",
                            engine=inst.engine,
                            ins=[],
                            outs=[],
                            sync_info=mybir.SyncInfo(on_wait=[w], on_update=[]),
                        )
                        insts.insert(idx, nop)
                        idx += 1
                        n += 1
                idx += 1

    _built["nc"] = nc
    return nc


def _bf16(a):
    import ml_dtypes

    return np.asarray(a, np.float32).astype(ml_dtypes.bfloat16)


def _pack_rows(mat):
    """[R, C] -> [ceil(R/128), 128, C] zero-padded row chunks."""
    R, C = mat.shape
    nch = -(-R // 128)
    out = np.zeros((nch * 128, C), mat.dtype)
    out[:R] = mat
    return np.ascontiguousarray(out.reshape(nch, 128, C))


def kernel(x1, x2, x3, x4, params):
    from concourse.bass_utils import run_bass_kernel_spmd

    p = params
    xs = [np.asarray(x) for x in (x1, x2, x3, x4)]
    nc = _build()

    X = np.concatenate(xs, axis=-1)  # [B, NSEQ, CH]
    Xh = X.reshape(B, H, NTOK, CH)

    g_kv = np.asarray(p["ln_kv_g"], np.float64)
    b_kv = np.asarray(p["ln_kv_b"], np.float64)
    Wk = np.asarray(p["Wk"], np.float64)
    Wv = np.asarray(p["Wv"], np.float64)
    inv = 1.0 / np.sqrt(CH)

    in_maps = []
    for h in range(H):
        m = {}
        m["ident"] = _bf16(np.eye(128, dtype=np.float32))
        m["x"] = np.ascontiguousarray(Xh[:, h])
        Wcomb = Wv[h] @ Wk[h]  # [d, ch]
        wkv = np.concatenate([(Wcomb * g_kv[None, :]).T, (Wcomb @ b_kv)[None, :]], axis=0)
        m["wkv"] = _bf16(_pack_rows(wkv))  # [8, 128, CH]
        for i, c in enumerate(CS):
            g1 = np.asarray(p[f"ln1_{i+1}_g"], np.float64)
            b1 = np.asarray(p[f"ln1_{i+1}_b"], np.float64)
            Wq = np.asarray(p[f"Wq{i+1}"], np.float64)[h] * inv
            wq = np.concatenate([(Wq * g1[None, :]).T, (Wq @ b1)[None, :]], axis=0)
            m[f"wq{i}"] = _bf16(_pack_rows(wq))
            Wo = np.asarray(p[f"Wo{i+1}"], np.float64)
            m[f"wo{i}"] = _bf16(_pack_rows(Wo.T))
            g2 = np.asarray(p[f"ln2_{i+1}_g"], np.float64)
            b2 = np.asarray(p[f"ln2_{i+1}_b"], np.float64)
            F1 = np.asarray(p[f"fc1_{i+1}_w"], np.float64)
            F1b = np.asarray(p[f"fc1_{i+1}_b"], np.float64)
            wf1 = np.concatenate([(F1 * g2[None, :]).T, (F1 @ b2 + F1b)[None, :]], axis=0)
            m[f"wf1{i}"] = _bf16(_pack_rows(wf1))
            F2 = np.asarray(p[f"fc2_{i+1}_w"], np.float64)
            F2b = np.asarray(p[f"fc2_{i+1}_b"], np.float64)
            wf2 = np.concatenate([F2.T, F2b[None, :]], axis=0)
            m[f"wf2{i}"] = _bf16(_pack_rows(wf2))
        in_maps.append(m)

    trace = os.environ.get("TRN_KERNEL_TRACE", "0") == "1"
    res = run_bass_kernel_spmd(nc, in_maps, core_ids=list(range(H)), trace=trace)
    LAST_EXEC_NS[0] = res.exec_time_ns

    outs = np.stack([res.results[h]["out"] for h in range(H)], axis=1)  # [B, H, NTOK, CH]
    full = outs.reshape(B, NSEQ, CH)
    return tuple(
        np.ascontiguousarray(full[:, :, COFF[i] : COFF[i] + CS[i]]) for i in range(4)
    )
